# revision 8
# baseline (speedup 1.0000x reference)
"""Trainium2 Bass kernel for ContactPositionRegressor (segment attention-pool + MLP head).

Contract: kernel(**inputs) takes the FULL unsharded inputs (as produced by the
problem's setup_inputs()) and returns the FULL [16, 3] fp32 output.

Strategy (data-parallel over point clouds, per the sharding hint):
  - 16 segments, 8 cores -> each core owns 2 whole segments ("slots").
  - Host slices the big [131072, 512] feature tensors by segment boundaries
    (zero-copy views in the uniform case) and replicates the small weights.
  - On each core, a single Bass/Tile program streams its feature rows once
    from HBM in 128-row chunks and computes, per chunk:
      PE transpose -> hidden matmul (feat @ w1) -> ReLU -> score matmul (@ w2)
      -> exp -> PE-accumulated exp-weighted feature sum + exp sum (softmax
      denominator), accumulated in PSUM across the whole segment.
    Softmax max-subtraction is skipped: weights are ~0.02-scale so scores are
    O(1) and exp() cannot overflow; the softmax ratio is unchanged.
  - The tiny MLP head (2 rows per core) runs on-device after the pools.
  - Host gathers the per-core [2, 3] outputs into [16, 3].

Ragged offsets are supported: segments are padded to a uniform slot size and
partially-valid 128-row chunks get 0/1 masks multiplied into the exp weights.
With the uniform reference offsets no masks are emitted at all.
"""

import os
import sys
import subprocess
import tempfile

import numpy as np

# ---- problem constants (hardcoded; kernel.py must be self-contained) ----
B = 16           # segments (point clouds)
C = 512          # feature channels
H = 128          # attention hidden dim (C // 4)
EMB = 32         # category embedding dim
H1, H2, NOUT = 256, 128, 3
FUSE = 2 * C + EMB   # 1056
NCORES = 8
SLOTS = 2        # segments per core
LN_EPS = 1e-5
ZCLAMP = 1e-37   # empty-segment guard for the softmax denominator

_WNAMES = (
    "att_w1", "att_b1", "att_w2",
    "patt_w1", "patt_b1", "patt_w2",
    "rw1", "rb1", "ln1_g", "ln1_b",
    "rw2", "rb2", "ln2_g", "ln2_b",
    "rw3", "rb3",
)

_PROG_CACHE = {}


def _build_program(S_pad, masked_key):
    """Build (and bass-compile) the per-core SPMD program.

    masked_key: sorted tuple of (feat_type, slot, chunk) whose 128-row chunk
    contains padding on at least one core; those chunks multiply the exp
    weights by a host-supplied 0/1 mask row.
    """
    from contextlib import ExitStack

    import concourse.bass as bass
    import concourse.tile as tile
    from concourse import bacc, mybir
    from concourse.masks import make_identity

    f32 = mybir.dt.float32
    AF = mybir.ActivationFunctionType
    ALU = mybir.AluOpType

    masked = {key: i for i, key in enumerate(masked_key)}
    n_masked = len(masked_key)
    NCH = S_pad // 128

    nc = bacc.Bacc("TRN2", target_bir_lowering=False, debug=False)

    lf = nc.dram_tensor("lf", [SLOTS * S_pad, C], f32, kind="ExternalInput")
    pf = nc.dram_tensor("pf", [SLOTS * S_pad, C], f32, kind="ExternalInput")
    embd = nc.dram_tensor("emb", [SLOTS, EMB], f32, kind="ExternalInput")
    aw1 = nc.dram_tensor("att_w1", [C, H], f32, kind="ExternalInput")
    ab1 = nc.dram_tensor("att_b1", [H], f32, kind="ExternalInput")
    aw2 = nc.dram_tensor("att_w2", [H], f32, kind="ExternalInput")
    pw1 = nc.dram_tensor("patt_w1", [C, H], f32, kind="ExternalInput")
    pb1 = nc.dram_tensor("patt_b1", [H], f32, kind="ExternalInput")
    pw2 = nc.dram_tensor("patt_w2", [H], f32, kind="ExternalInput")
    rw1d = nc.dram_tensor("rw1", [FUSE, H1], f32, kind="ExternalInput")
    rb1d = nc.dram_tensor("rb1", [H1], f32, kind="ExternalInput")
    g1d = nc.dram_tensor("ln1_g", [H1], f32, kind="ExternalInput")
    be1d = nc.dram_tensor("ln1_b", [H1], f32, kind="ExternalInput")
    rw2d = nc.dram_tensor("rw2", [H1, H2], f32, kind="ExternalInput")
    rb2d = nc.dram_tensor("rb2", [H2], f32, kind="ExternalInput")
    g2d = nc.dram_tensor("ln2_g", [H2], f32, kind="ExternalInput")
    be2d = nc.dram_tensor("ln2_b", [H2], f32, kind="ExternalInput")
    rw3d = nc.dram_tensor("rw3", [H2, NOUT], f32, kind="ExternalInput")
    rb3d = nc.dram_tensor("rb3", [NOUT], f32, kind="ExternalInput")
    maskd = None
    if n_masked:
        maskd = nc.dram_tensor("maskrows", [n_masked, 128], f32, kind="ExternalInput")
    out_d = nc.dram_tensor("out", [SLOTS, NOUT], f32, kind="ExternalOutput")

    with tile.TileContext(nc) as tc:
        with ExitStack() as octx:
            const = octx.enter_context(tc.tile_pool(name="const", bufs=1))
            persist = octx.enter_context(tc.tile_pool(name="persist", bufs=1))

            identity = const.tile([128, 128], f32, tag="identity")
            make_identity(nc, identity[:])
            ones = const.tile([128, 1], f32, tag="ones")
            nc.vector.memset(ones, 1.0)

            w1s, b1s, w2s = [], [], []
            for t, (w1d, b1d, w2d) in enumerate(((aw1, ab1, aw2), (pw1, pb1, pw2))):
                w1t = const.tile([128, 4, H], f32, tag=f"w1_{t}")
                for k in range(4):
                    nc.sync.dma_start(out=w1t[:, k, :], in_=w1d[128 * k:128 * (k + 1), :])
                b1t = const.tile([128, 1], f32, tag=f"b1_{t}")
                nc.gpsimd.dma_start(out=b1t, in_=b1d[:].rearrange("(p o) -> p o", o=1))
                w2t = const.tile([128, 1], f32, tag=f"w2_{t}")
                nc.gpsimd.dma_start(out=w2t, in_=w2d[:].rearrange("(p o) -> p o", o=1))
                w1s.append(w1t)
                b1s.append(b1t)
                w2s.append(w2t)

            g_sb = persist.tile([SLOTS, FUSE], f32, tag="g")
            zcol = persist.tile([SLOTS, 2], f32, tag="zcol")

            # ---------------- main streaming loop ----------------
            with ExitStack() as mctx:
                featp = mctx.enter_context(tc.tile_pool(name="featp", bufs=8))
                featTp = mctx.enter_context(tc.tile_pool(name="featTp", bufs=3))
                relup = mctx.enter_context(tc.tile_pool(name="relup", bufs=3))
                ep = mctx.enter_context(tc.tile_pool(name="ep", bufs=4))
                smallp = mctx.enter_context(tc.tile_pool(name="smallp", bufs=2))
                tpsum = mctx.enter_context(tc.tile_pool(name="tpsum", bufs=2, space="PSUM"))
                hpsum = mctx.enter_context(tc.tile_pool(name="hpsum", bufs=2, space="PSUM"))
                spsum = mctx.enter_context(tc.tile_pool(name="spsum", bufs=2, space="PSUM"))
                paccp = mctx.enter_context(tc.tile_pool(name="paccp", bufs=1, space="PSUM"))
                zaccp = mctx.enter_context(tc.tile_pool(name="zaccp", bufs=1, space="PSUM"))

                for t, fd in ((0, lf), (1, pf)):
                    for slot in range(SLOTS):
                        pacc = paccp.tile([1, C], f32, tag="pacc")
                        zacc = zaccp.tile([1, 1], f32, tag="zacc")
                        for ch in range(NCH):
                            feat = featp.tile([128, C], f32, tag="feat")
                            row0 = slot * S_pad + ch * 128
                            nc.sync.dma_start(out=feat, in_=fd[row0:row0 + 128, :])

                            pt = tpsum.tile([128, C], f32, tag="pt")
                            for k in range(4):
                                nc.tensor.transpose(
                                    pt[:, 128 * k:128 * (k + 1)],
                                    feat[:, 128 * k:128 * (k + 1)],
                                    identity[:],
                                )
                            ft = featTp.tile([128, C], f32, tag="ft")
                            nc.vector.tensor_copy(ft, pt)

                            hp_ = hpsum.tile([128, 128], f32, tag="hp")
                            for k in range(4):
                                nc.tensor.matmul(
                                    hp_,
                                    w1s[t][:, k, :],
                                    ft[:, 128 * k:128 * (k + 1)],
                                    start=(k == 0),
                                    stop=(k == 3),
                                )
                            rl = relup.tile([128, 128], f32, tag="rl")
                            nc.scalar.activation(rl, hp_, AF.Relu, bias=b1s[t])

                            sp = spsum.tile([128, 1], f32, tag="sp")
                            nc.tensor.matmul(sp, rl, w2s[t], start=True, stop=True)

                            e = ep.tile([128, 1], f32, tag="e")
                            nc.scalar.activation(e, sp, AF.Exp)

                            mi = masked.get((t, slot, ch))
                            if mi is not None:
                                mt = ep.tile([128, 1], f32, tag="mask")
                                nc.gpsimd.dma_start(
                                    out=mt,
                                    in_=maskd[mi:mi + 1, :].rearrange("o p -> p o"),
                                )
                                em = ep.tile([128, 1], f32, tag="em")
                                nc.vector.tensor_mul(em, e, mt)
                                e = em

                            nc.tensor.matmul(pacc, e, feat, start=(ch == 0), stop=(ch == NCH - 1))
                            nc.tensor.matmul(zacc, e, ones, start=(ch == 0), stop=(ch == NCH - 1))

                        pooled = smallp.tile([1, C], f32, tag="pooled")
                        nc.scalar.copy(pooled, pacc)
                        zsb = smallp.tile([1, 1], f32, tag="zsb")
                        nc.vector.tensor_scalar_max(zsb, zacc, ZCLAMP)
                        nc.sync.dma_start(out=g_sb[slot:slot + 1, t * C:(t + 1) * C], in_=pooled)
                        nc.sync.dma_start(out=zcol[slot:slot + 1, t:t + 1], in_=zsb)

            # ---------------- MLP head (2 rows) ----------------
            with ExitStack() as hctx:
                hsb = hctx.enter_context(tc.tile_pool(name="hsb", bufs=2))
                hpsm = hctx.enter_context(tc.tile_pool(name="hpsm", bufs=2, space="PSUM"))

                nc.sync.dma_start(out=g_sb[:, 2 * C:2 * C + EMB], in_=embd[:, :])

                zinv = hsb.tile([SLOTS, 2], f32, tag="zinv")
                nc.vector.reciprocal(zinv, zcol)
                nc.vector.tensor_scalar_mul(g_sb[:, 0:C], g_sb[:, 0:C], zinv[:, 0:1])
                nc.vector.tensor_scalar_mul(g_sb[:, C:2 * C], g_sb[:, C:2 * C], zinv[:, 1:2])

                def bload(drt, n, tag):
                    tb = hsb.tile([SLOTS, n], f32, tag=tag)
                    src = drt[:]
                    bcast = bass.AP(
                        tensor=src.tensor, offset=src.offset,
                        ap=[[0, SLOTS]] + list(src.ap),
                    )
                    nc.gpsimd.dma_start(out=tb, in_=bcast)
                    return tb

                rb1b = bload(rb1d, H1, "rb1b")
                g1b = bload(g1d, H1, "g1b")
                be1b = bload(be1d, H1, "be1b")
                rb2b = bload(rb2d, H2, "rb2b")
                g2b = bload(g2d, H2, "g2b")
                be2b = bload(be2d, H2, "be2b")
                rb3b = bload(rb3d, NOUT, "rb3b")
                epsb = hsb.tile([SLOTS, 1], f32, tag="epsb")
                nc.vector.memset(epsb, LN_EPS)

                n_k1 = (FUSE + 127) // 128  # 9
                rw1t = const.tile([128, n_k1, H1], f32, tag="rw1t")
                for k in range(n_k1):
                    kk = min(128, FUSE - 128 * k)
                    nc.sync.dma_start(out=rw1t[:kk, k, :], in_=rw1d[128 * k:128 * k + kk, :])
                rw2t = const.tile([128, 2, H2], f32, tag="rw2t")
                for k in range(2):
                    nc.sync.dma_start(out=rw2t[:, k, :], in_=rw2d[128 * k:128 * (k + 1), :])
                rw3t = const.tile([128, NOUT], f32, tag="rw3t")
                nc.sync.dma_start(out=rw3t, in_=rw3d[:, :])

                def ln_relu(x, n, gam, bet, idx):
                    stats = hsb.tile([SLOTS, 6], f32, tag=f"stats{idx}")
                    nc.vector.bn_stats(out=stats, in_=x)
                    mv = hsb.tile([SLOTS, 2], f32, tag=f"mv{idx}")
                    nc.vector.bn_aggr(out=mv, in_=stats)
                    std = hsb.tile([SLOTS, 1], f32, tag=f"std{idx}")
                    nc.scalar.activation(std, mv[:, 1:2], AF.Sqrt, bias=epsb)
                    rstd = hsb.tile([SLOTS, 1], f32, tag=f"rstd{idx}")
                    nc.vector.reciprocal(rstd, std)
                    nc.vector.tensor_scalar(
                        out=x, in0=x,
                        scalar1=mv[:, 0:1], scalar2=rstd,
                        op0=ALU.subtract, op1=ALU.mult,
                    )
                    nc.vector.tensor_mul(x, x, gam)
                    nc.vector.tensor_add(x, x, bet)
                    nc.vector.tensor_scalar_max(x, x, 0.0)

                # h1 = relu(LN(g @ rw1 + rb1))
                gT = hsb.tile([128, n_k1, SLOTS], f32, tag="gT")
                for k in range(n_k1):
                    kk = min(128, FUSE - 128 * k)
                    ptg = hpsm.tile([128, SLOTS], f32, tag="ptx")
                    nc.tensor.transpose(
                        ptg[:kk, :], g_sb[:, 128 * k:128 * k + kk], identity[:SLOTS, :SLOTS]
                    )
                    nc.vector.tensor_copy(gT[:kk, k, :], ptg[:kk, :])
                h1ps = hpsm.tile([SLOTS, H1], f32, tag="hacc")
                for k in range(n_k1):
                    kk = min(128, FUSE - 128 * k)
                    nc.tensor.matmul(
                        h1ps, gT[:kk, k, :], rw1t[:kk, k, :],
                        start=(k == 0), stop=(k == n_k1 - 1),
                    )
                h1 = hsb.tile([SLOTS, H1], f32, tag="h1")
                nc.vector.tensor_add(h1, h1ps, rb1b)
                ln_relu(h1, H1, g1b, be1b, 1)

                # h2 = relu(LN(h1 @ rw2 + rb2))
                h1T = hsb.tile([128, 2, SLOTS], f32, tag="h1T")
                for k in range(2):
                    pth = hpsm.tile([128, SLOTS], f32, tag="ptx")
                    nc.tensor.transpose(
                        pth[:, :], h1[:, 128 * k:128 * (k + 1)], identity[:SLOTS, :SLOTS]
                    )
                    nc.vector.tensor_copy(h1T[:, k, :], pth[:, :])
                h2ps = hpsm.tile([SLOTS, H2], f32, tag="hacc")
                for k in range(2):
                    nc.tensor.matmul(
                        h2ps, h1T[:, k, :], rw2t[:, k, :],
                        start=(k == 0), stop=(k == 1),
                    )
                h2 = hsb.tile([SLOTS, H2], f32, tag="h2")
                nc.vector.tensor_add(h2, h2ps, rb2b)
                ln_relu(h2, H2, g2b, be2b, 2)

                # out = h2 @ rw3 + rb3
                h2T = hsb.tile([128, SLOTS], f32, tag="h2T")
                pto = hpsm.tile([128, SLOTS], f32, tag="ptx")
                nc.tensor.transpose(pto[:, :], h2[:, :], identity[:SLOTS, :SLOTS])
                nc.vector.tensor_copy(h2T, pto)
                ops_ = hpsm.tile([SLOTS, NOUT], f32, tag="hacc")
                nc.tensor.matmul(ops_, h2T, rw3t, start=True, stop=True)
                osb = hsb.tile([SLOTS, NOUT], f32, tag="osb")
                nc.vector.tensor_add(osb, ops_, rb3b)
                nc.sync.dma_start(out=out_d[:, :], in_=osb)

    nc.compile()
    return nc


def _get_program(S_pad, masked_key):
    key = (S_pad, masked_key)
    if key not in _PROG_CACHE:
        _PROG_CACHE[key] = _build_program(S_pad, masked_key)
    return _PROG_CACHE[key]


def _segment_bounds(offset):
    off = np.asarray(offset).astype(np.int64)
    start = np.concatenate([[0], off[:-1]])
    return start, off - start


def _core_feat(feat, start, lens, core, S_pad):
    """Rows for this core's SLOTS segments, padded to [SLOTS * S_pad, C]."""
    segs = range(core * SLOTS, (core + 1) * SLOTS)
    if all(
        lens[j] == S_pad and start[j] == j * S_pad
        for j in segs
    ):
        return feat[core * SLOTS * S_pad:(core + 1) * SLOTS * S_pad]
    buf = np.zeros((SLOTS * S_pad, C), np.float32)
    for s, j in enumerate(segs):
        L = int(lens[j])
        if L > 0:
            buf[s * S_pad:s * S_pad + L] = feat[int(start[j]):int(start[j]) + L]
    return buf


def _prepare(inputs):
    lfeat = np.ascontiguousarray(np.asarray(inputs["local_feat"]), dtype=np.float32)
    pfeat = np.ascontiguousarray(np.asarray(inputs["parent_feat"]), dtype=np.float32)
    lstart, llen = _segment_bounds(inputs["local_offset"])
    pstart, plen = _segment_bounds(inputs["parent_offset"])
    cid = np.asarray(inputs["category_id"]).astype(np.int64)
    cat_emb = np.asarray(inputs["cat_emb"], dtype=np.float32)
    assert llen.shape[0] == B and plen.shape[0] == B

    S_pad = int(max(llen.max(), plen.max(), 1))
    S_pad = ((S_pad + 127) // 128) * 128
    NCH = S_pad // 128

    masked = set()
    for t, lens in ((0, llen), (1, plen)):
        for j in range(B):
            L = int(lens[j])
            slot = j % SLOTS
            for ch in range(NCH):
                v = min(128, max(0, L - ch * 128))
                if v < 128:
                    masked.add((t, slot, ch))
    masked_key = tuple(sorted(masked))

    weights = {k: np.ascontiguousarray(np.asarray(inputs[k]), dtype=np.float32)
               for k in _WNAMES}
    in_maps = []
    for core in range(NCORES):
        m = dict(weights)
        m["lf"] = _core_feat(lfeat, lstart, llen, core, S_pad)
        m["pf"] = _core_feat(pfeat, pstart, plen, core, S_pad)
        m["emb"] = np.ascontiguousarray(
            cat_emb[cid[core * SLOTS:(core + 1) * SLOTS]], dtype=np.float32)
        if masked_key:
            rows = np.zeros((len(masked_key), 128), np.float32)
            for i, (t, slot, ch) in enumerate(masked_key):
                j = core * SLOTS + slot
                L = int((llen if t == 0 else plen)[j])
                v = min(128, max(0, L - ch * 128))
                rows[i, :v] = 1.0
            m["maskrows"] = rows
        in_maps.append(m)
    return S_pad, masked_key, in_maps


def _ensure_ntff_hook():
    """Register the axon NTFF profiling hook if the antenv shim module is
    missing (trimmed container). Profiling-only; the grading path never needs it."""
    import types
    try:
        from antenv import axon_hooks  # noqa: F401
        return
    except ImportError:
        pass
    try:
        import antenv
        mod = types.ModuleType("antenv.axon_hooks")
        _h = {"hook": None}
        mod.set_axon_ntff_profile_hook = lambda h: _h.__setitem__("hook", h)
        mod.get_axon_ntff_profile_hook = lambda: _h["hook"]
        sys.modules["antenv.axon_hooks"] = mod
        antenv.axon_hooks = mod
        from trn_agent_boot.trn_boot import _ntff_profile_via_ctypes
        hook = _ntff_profile_via_ctypes("/opt/axon/libaxon_pjrt.so")
        if hook is not None:
            mod.set_axon_ntff_profile_hook(hook)
    except Exception:
        pass


def _run_inproc(inputs, trace=False):
    from concourse import bass_utils
    if trace:
        _ensure_ntff_hook()
    S_pad, masked_key, in_maps = _prepare(inputs)
    nc = _get_program(S_pad, masked_key)
    bkr = bass_utils.run_bass_kernel_spmd(
        nc, in_maps, core_ids=list(range(NCORES)), trace=trace)
    out = np.concatenate(
        [np.asarray(r["out"], np.float32).reshape(SLOTS, NOUT) for r in bkr.results],
        axis=0,
    )
    return out, bkr


def _jax_platform_ok():
    """The device run needs a non-CPU (axon/neuron) PJRT backend in-process."""
    if "jax" not in sys.modules:
        os.environ.setdefault("JAX_PLATFORMS", "axon")
    try:
        import jax
        return any(d.platform != "cpu" for d in jax.devices())
    except Exception:
        return False


def _run_subproc(inputs):
    """Fallback: run the device part in a child process with the axon backend."""
    with tempfile.TemporaryDirectory() as td:
        inp = os.path.join(td, "in.npz")
        outp = os.path.join(td, "out.npy")
        np.savez(inp, **{k: np.asarray(v) for k, v in inputs.items()})
        driver = (
            "import os, sys, numpy as np\n"
            f"sys.path.insert(0, {os.path.dirname(os.path.abspath(__file__))!r})\n"
            "import kernel as K\n"
            f"d = dict(np.load({inp!r}))\n"
            f"out, _ = K._run_inproc(d)\n"
            f"np.save({outp!r}, out)\n"
        )
        env = dict(os.environ)
        env["JAX_PLATFORMS"] = "axon"
        subprocess.run([sys.executable, "-c", driver], env=env, check=True)
        return np.load(outp)


def kernel(**inputs):
    # The subprocess path cannot recurse: the child calls _run_inproc directly.
    # It exists only for host processes whose jax is pinned to CPU.
    if _jax_platform_ok():
        return _run_inproc(inputs)[0]
    return _run_subproc(inputs)


if __name__ == "__main__":
    # smoke test with tiny random data (ragged offsets exercise the mask path)
    rng = np.random.default_rng(0)
    Npts = 16 * 320
    off = np.sort(rng.choice(np.arange(1, Npts), size=B - 1, replace=False))
    off = np.concatenate([off, [Npts]]).astype(np.int32)
    inp = {
        "local_feat": rng.standard_normal((Npts, C), dtype=np.float32),
        "parent_feat": rng.standard_normal((Npts, C), dtype=np.float32),
        "local_offset": off,
        "parent_offset": off,
        "category_id": rng.integers(0, 3, B).astype(np.int32),
        "att_w1": 0.02 * rng.standard_normal((C, H), dtype=np.float32),
        "att_b1": 0.01 * rng.standard_normal(H).astype(np.float32),
        "att_w2": 0.02 * rng.standard_normal(H).astype(np.float32),
        "patt_w1": 0.02 * rng.standard_normal((C, H), dtype=np.float32),
        "patt_b1": 0.01 * rng.standard_normal(H).astype(np.float32),
        "patt_w2": 0.02 * rng.standard_normal(H).astype(np.float32),
        "cat_emb": rng.standard_normal((3, EMB), dtype=np.float32),
        "rw1": 0.02 * rng.standard_normal((FUSE, H1), dtype=np.float32),
        "rb1": 0.01 * rng.standard_normal(H1).astype(np.float32),
        "ln1_g": (1 + 0.1 * rng.standard_normal(H1)).astype(np.float32),
        "ln1_b": 0.1 * rng.standard_normal(H1).astype(np.float32),
        "rw2": 0.02 * rng.standard_normal((H1, H2), dtype=np.float32),
        "rb2": 0.01 * rng.standard_normal(H2).astype(np.float32),
        "ln2_g": (1 + 0.1 * rng.standard_normal(H2)).astype(np.float32),
        "ln2_b": 0.1 * rng.standard_normal(H2).astype(np.float32),
        "rw3": 0.02 * rng.standard_normal((H2, NOUT), dtype=np.float32),
        "rb3": 0.01 * rng.standard_normal(NOUT).astype(np.float32),
    }
    got = kernel(**inp)

    # numpy reference
    def np_ref(i):
        def pool(feat, off, w1, b1, w2):
            start = 0
            outs = []
            for j in range(B):
                seg = feat[start:off[j]]
                start = off[j]
                s = np.maximum(seg @ w1 + b1, 0) @ w2
                e = np.exp(s - (s.max() if s.size else 0.0))
                outs.append((seg * e[:, None]).sum(0) / max(e.sum(), 1e-37)
                            if s.size else np.zeros(C, np.float32))
            return np.stack(outs)

        lg = pool(i["local_feat"], i["local_offset"], i["att_w1"], i["att_b1"], i["att_w2"])
        pg = pool(i["parent_feat"], i["parent_offset"], i["patt_w1"], i["patt_b1"], i["patt_w2"])
        g = np.concatenate([lg, pg, i["cat_emb"][i["category_id"]]], -1)

        def lnr(x, gm, bt):
            mu = x.mean(-1, keepdims=True)
            v = x.var(-1, keepdims=True)
            return np.maximum((x - mu) / np.sqrt(v + LN_EPS) * gm + bt, 0)

        h = lnr(g @ i["rw1"] + i["rb1"], i["ln1_g"], i["ln1_b"])
        h = lnr(h @ i["rw2"] + i["rb2"], i["ln2_g"], i["ln2_b"])
        return h @ i["rw3"] + i["rb3"]

    want = np_ref(inp)
    err = np.max(np.abs(got - want)) / max(np.max(np.abs(want)), 1e-30)
    print("smoke got[0]:", got[0], " want[0]:", want[0])
    print("smoke rel err:", err)


# revision 10
# speedup vs baseline: 1.2129x; 1.2129x over previous
"""Trainium2 Bass kernel for ContactPositionRegressor (segment attention-pool + MLP head).

Contract: kernel(**inputs) takes the FULL unsharded inputs (as produced by the
problem's setup_inputs()) and returns the FULL [16, 3] fp32 output.

Strategy (data-parallel over point clouds, per the sharding hint):
  - 16 segments, 8 cores -> each core owns 2 whole segments ("slots").
  - Host slices the big [131072, 512] feature tensors by segment boundaries
    (zero-copy views in the uniform case) and replicates the small weights.
  - On each core, a single Bass/Tile program streams its feature rows once
    from HBM in 128-row chunks and computes, per chunk:
      PE transpose -> hidden matmul (feat @ w1) -> ReLU -> score matmul (@ w2)
      -> exp -> PE-accumulated exp-weighted feature sum + exp sum (softmax
      denominator), accumulated in PSUM across the whole segment.
    Softmax max-subtraction is skipped: weights are ~0.02-scale so scores are
    O(1) and exp() cannot overflow; the softmax ratio is unchanged.
  - The tiny MLP head (2 rows per core) runs on-device after the pools.
  - Host gathers the per-core [2, 3] outputs into [16, 3].

Ragged offsets are supported: segments are padded to a uniform slot size and
partially-valid 128-row chunks get 0/1 masks multiplied into the exp weights.
With the uniform reference offsets no masks are emitted at all.
"""

import os
import sys
import subprocess
import tempfile

import numpy as np

# ---- problem constants (hardcoded; kernel.py must be self-contained) ----
B = 16           # segments (point clouds)
C = 512          # feature channels
H = 128          # attention hidden dim (C // 4)
EMB = 32         # category embedding dim
H1, H2, NOUT = 256, 128, 3
FUSE = 2 * C + EMB   # 1056
NCORES = 8
SLOTS = 2        # segments per core
LN_EPS = 1e-5
ZCLAMP = 1e-37   # empty-segment guard for the softmax denominator

_WNAMES = (
    "att_w1", "att_b1", "att_w2",
    "patt_w1", "patt_b1", "patt_w2",
    "rw1", "rb1", "ln1_g", "ln1_b",
    "rw2", "rb2", "ln2_g", "ln2_b",
    "rw3", "rb3",
)

_PROG_CACHE = {}


def _build_program(S_pad, masked_key):
    """Build (and bass-compile) the per-core SPMD program.

    masked_key: sorted tuple of (feat_type, slot, chunk) whose 128-row chunk
    contains padding on at least one core; those chunks multiply the exp
    weights by a host-supplied 0/1 mask row.
    """
    from contextlib import ExitStack

    import concourse.bass as bass
    import concourse.tile as tile
    from concourse import bacc, mybir
    from concourse.masks import make_identity

    f32 = mybir.dt.float32
    AF = mybir.ActivationFunctionType
    ALU = mybir.AluOpType

    masked = {key: i for i, key in enumerate(masked_key)}
    n_masked = len(masked_key)
    NCH = S_pad // 128

    nc = bacc.Bacc("TRN2", target_bir_lowering=False, debug=False)

    lf = nc.dram_tensor("lf", [SLOTS * S_pad, C], f32, kind="ExternalInput")
    pf = nc.dram_tensor("pf", [SLOTS * S_pad, C], f32, kind="ExternalInput")
    embd = nc.dram_tensor("emb", [SLOTS, EMB], f32, kind="ExternalInput")
    aw1 = nc.dram_tensor("att_w1", [C, H], f32, kind="ExternalInput")
    ab1 = nc.dram_tensor("att_b1", [H], f32, kind="ExternalInput")
    aw2 = nc.dram_tensor("att_w2", [H], f32, kind="ExternalInput")
    pw1 = nc.dram_tensor("patt_w1", [C, H], f32, kind="ExternalInput")
    pb1 = nc.dram_tensor("patt_b1", [H], f32, kind="ExternalInput")
    pw2 = nc.dram_tensor("patt_w2", [H], f32, kind="ExternalInput")
    rw1d = nc.dram_tensor("rw1", [FUSE, H1], f32, kind="ExternalInput")
    rb1d = nc.dram_tensor("rb1", [H1], f32, kind="ExternalInput")
    g1d = nc.dram_tensor("ln1_g", [H1], f32, kind="ExternalInput")
    be1d = nc.dram_tensor("ln1_b", [H1], f32, kind="ExternalInput")
    rw2d = nc.dram_tensor("rw2", [H1, H2], f32, kind="ExternalInput")
    rb2d = nc.dram_tensor("rb2", [H2], f32, kind="ExternalInput")
    g2d = nc.dram_tensor("ln2_g", [H2], f32, kind="ExternalInput")
    be2d = nc.dram_tensor("ln2_b", [H2], f32, kind="ExternalInput")
    rw3d = nc.dram_tensor("rw3", [H2, NOUT], f32, kind="ExternalInput")
    rb3d = nc.dram_tensor("rb3", [NOUT], f32, kind="ExternalInput")
    maskd = None
    if n_masked:
        maskd = nc.dram_tensor("maskrows", [n_masked, 128], f32, kind="ExternalInput")
    out_d = nc.dram_tensor("out", [SLOTS, NOUT], f32, kind="ExternalOutput")

    with tile.TileContext(nc) as tc:
        with ExitStack() as octx:
            const = octx.enter_context(tc.tile_pool(name="const", bufs=1))
            persist = octx.enter_context(tc.tile_pool(name="persist", bufs=1))

            identity = const.tile([128, 128], f32, tag="identity")
            make_identity(nc, identity[:])
            ones = const.tile([128, 1], f32, tag="ones")
            nc.vector.memset(ones, 1.0)

            w1s, b1s, w2s = [], [], []
            for t, (w1d, b1d, w2d) in enumerate(((aw1, ab1, aw2), (pw1, pb1, pw2))):
                w1t = const.tile([128, 4, H], f32, tag=f"w1_{t}")
                for k in range(4):
                    nc.sync.dma_start(out=w1t[:, k, :], in_=w1d[128 * k:128 * (k + 1), :])
                b1t = const.tile([128, 1], f32, tag=f"b1_{t}")
                nc.gpsimd.dma_start(out=b1t, in_=b1d[:].rearrange("(p o) -> p o", o=1))
                w2t = const.tile([128, 1], f32, tag=f"w2_{t}")
                nc.gpsimd.dma_start(out=w2t, in_=w2d[:].rearrange("(p o) -> p o", o=1))
                w1s.append(w1t)
                b1s.append(b1t)
                w2s.append(w2t)

            g_sb = persist.tile([SLOTS, FUSE], f32, tag="g")
            zcol = persist.tile([SLOTS, 2], f32, tag="zcol")

            # ---------------- main streaming loop (software-pipelined) ----------------
            # PE executes in emission order; the naive per-chunk chain stalls it
            # on DVE/ACT round-trips three times per chunk. Skew the stages so
            # every PE op consumes results produced >= 1 full iteration earlier:
            #   iter i:  transpose(i) | hidden+relu(i-1) | score+exp(i-2) | pool/z(i-3)
            with ExitStack() as mctx:
                featp = mctx.enter_context(tc.tile_pool(name="featp", bufs=8))
                featTp = mctx.enter_context(tc.tile_pool(name="featTp", bufs=3))
                relup = mctx.enter_context(tc.tile_pool(name="relup", bufs=3))
                ep = mctx.enter_context(tc.tile_pool(name="ep", bufs=4))
                smallp = mctx.enter_context(tc.tile_pool(name="smallp", bufs=2))
                tpsum = mctx.enter_context(tc.tile_pool(name="tpsum", bufs=2, space="PSUM"))
                hpsum = mctx.enter_context(tc.tile_pool(name="hpsum", bufs=2, space="PSUM"))
                spsum = mctx.enter_context(tc.tile_pool(name="spsum", bufs=1, space="PSUM"))
                paccp = mctx.enter_context(tc.tile_pool(name="paccp", bufs=2, space="PSUM"))
                zaccp = mctx.enter_context(tc.tile_pool(name="zaccp", bufs=1, space="PSUM"))

                work = []
                for t, fd in ((0, lf), (1, pf)):
                    for slot in range(SLOTS):
                        for ch in range(NCH):
                            work.append((t, slot, ch, fd))
                n_work = len(work)
                SKEW_H, SKEW_S, SKEW_P = 1, 2, 3
                state = {}
                accs = {}

                for i in range(n_work + SKEW_P):
                    if i < n_work:
                        t, slot, ch, fd = work[i]
                        feat = featp.tile([128, C], f32, tag="feat")
                        row0 = slot * S_pad + ch * 128
                        nc.sync.dma_start(out=feat, in_=fd[row0:row0 + 128, :])
                        pt = tpsum.tile([128, C], f32, tag="pt")
                        for k in range(4):
                            nc.tensor.transpose(
                                pt[:, 128 * k:128 * (k + 1)],
                                feat[:, 128 * k:128 * (k + 1)],
                                identity[:],
                            )
                        ft = featTp.tile([128, C], f32, tag="ft")
                        nc.vector.tensor_copy(ft, pt)
                        state[i] = {"feat": feat, "ft": ft}

                    j = i - SKEW_H
                    if 0 <= j < n_work:
                        t, slot, ch, fd = work[j]
                        st = state[j]
                        hp_ = hpsum.tile([128, 128], f32, tag="hp")
                        for k in range(4):
                            nc.tensor.matmul(
                                hp_, w1s[t][:, k, :], st["ft"][:, 128 * k:128 * (k + 1)],
                                start=(k == 0), stop=(k == 3),
                            )
                        rl = relup.tile([128, 128], f32, tag="rl")
                        nc.scalar.activation(rl, hp_, AF.Relu, bias=b1s[t])
                        st["rl"] = rl

                    j = i - SKEW_S
                    if 0 <= j < n_work:
                        t, slot, ch, fd = work[j]
                        st = state[j]
                        sp = spsum.tile([128, 1], f32, tag="sp")
                        nc.tensor.matmul(sp, st["rl"], w2s[t], start=True, stop=True)
                        e = ep.tile([128, 1], f32, tag="e")
                        nc.scalar.activation(e, sp, AF.Exp)
                        mi = masked.get((t, slot, ch))
                        if mi is not None:
                            mt = ep.tile([128, 1], f32, tag="mask")
                            nc.gpsimd.dma_start(
                                out=mt,
                                in_=maskd[mi:mi + 1, :].rearrange("o p -> p o"),
                            )
                            em = ep.tile([128, 1], f32, tag="em")
                            nc.vector.tensor_mul(em, e, mt)
                            e = em
                        st["e"] = e

                    j = i - SKEW_P
                    if 0 <= j < n_work:
                        t, slot, ch, fd = work[j]
                        st = state.pop(j)
                        if ch == 0:
                            pacc_t = paccp.tile([1, C], f32, tag="pacc")
                            zacc_t = zaccp.tile([1, 1], f32, tag="zacc")
                            accs[(t, slot)] = (pacc_t, zacc_t)
                        pacc, zacc = accs[(t, slot)]
                        nc.tensor.matmul(pacc, st["e"], st["feat"],
                                         start=(ch == 0), stop=(ch == NCH - 1))
                        nc.tensor.matmul(zacc, st["e"], ones,
                                         start=(ch == 0), stop=(ch == NCH - 1))
                        if ch == NCH - 1:
                            del accs[(t, slot)]
                            pooled = smallp.tile([1, C], f32, tag="pooled")
                            nc.scalar.copy(pooled, pacc)
                            zsb = smallp.tile([1, 1], f32, tag="zsb")
                            nc.vector.tensor_scalar_max(zsb, zacc, ZCLAMP)
                            nc.sync.dma_start(
                                out=g_sb[slot:slot + 1, t * C:(t + 1) * C], in_=pooled)
                            nc.sync.dma_start(
                                out=zcol[slot:slot + 1, t:t + 1], in_=zsb)

            # ---------------- MLP head (2 rows) ----------------
            with ExitStack() as hctx:
                hsb = hctx.enter_context(tc.tile_pool(name="hsb", bufs=2))
                hpsm = hctx.enter_context(tc.tile_pool(name="hpsm", bufs=2, space="PSUM"))

                nc.sync.dma_start(out=g_sb[:, 2 * C:2 * C + EMB], in_=embd[:, :])

                zinv = hsb.tile([SLOTS, 2], f32, tag="zinv")
                nc.vector.reciprocal(zinv, zcol)
                nc.vector.tensor_scalar_mul(g_sb[:, 0:C], g_sb[:, 0:C], zinv[:, 0:1])
                nc.vector.tensor_scalar_mul(g_sb[:, C:2 * C], g_sb[:, C:2 * C], zinv[:, 1:2])

                def bload(drt, n, tag):
                    tb = hsb.tile([SLOTS, n], f32, tag=tag)
                    src = drt[:]
                    bcast = bass.AP(
                        tensor=src.tensor, offset=src.offset,
                        ap=[[0, SLOTS]] + list(src.ap),
                    )
                    nc.gpsimd.dma_start(out=tb, in_=bcast)
                    return tb

                rb1b = bload(rb1d, H1, "rb1b")
                g1b = bload(g1d, H1, "g1b")
                be1b = bload(be1d, H1, "be1b")
                rb2b = bload(rb2d, H2, "rb2b")
                g2b = bload(g2d, H2, "g2b")
                be2b = bload(be2d, H2, "be2b")
                rb3b = bload(rb3d, NOUT, "rb3b")
                epsb = hsb.tile([SLOTS, 1], f32, tag="epsb")
                nc.vector.memset(epsb, LN_EPS)

                n_k1 = (FUSE + 127) // 128  # 9
                rw1t = const.tile([128, n_k1, H1], f32, tag="rw1t")
                for k in range(n_k1):
                    kk = min(128, FUSE - 128 * k)
                    nc.sync.dma_start(out=rw1t[:kk, k, :], in_=rw1d[128 * k:128 * k + kk, :])
                rw2t = const.tile([128, 2, H2], f32, tag="rw2t")
                for k in range(2):
                    nc.sync.dma_start(out=rw2t[:, k, :], in_=rw2d[128 * k:128 * (k + 1), :])
                rw3t = const.tile([128, NOUT], f32, tag="rw3t")
                nc.sync.dma_start(out=rw3t, in_=rw3d[:, :])

                def ln_relu(x, n, gam, bet, idx):
                    stats = hsb.tile([SLOTS, 6], f32, tag=f"stats{idx}")
                    nc.vector.bn_stats(out=stats, in_=x)
                    mv = hsb.tile([SLOTS, 2], f32, tag=f"mv{idx}")
                    nc.vector.bn_aggr(out=mv, in_=stats)
                    std = hsb.tile([SLOTS, 1], f32, tag=f"std{idx}")
                    nc.scalar.activation(std, mv[:, 1:2], AF.Sqrt, bias=epsb)
                    rstd = hsb.tile([SLOTS, 1], f32, tag=f"rstd{idx}")
                    nc.vector.reciprocal(rstd, std)
                    nc.vector.tensor_scalar(
                        out=x, in0=x,
                        scalar1=mv[:, 0:1], scalar2=rstd,
                        op0=ALU.subtract, op1=ALU.mult,
                    )
                    nc.vector.tensor_mul(x, x, gam)
                    nc.vector.tensor_add(x, x, bet)
                    nc.vector.tensor_scalar_max(x, x, 0.0)

                # h1 = relu(LN(g @ rw1 + rb1))
                gT = hsb.tile([128, n_k1, SLOTS], f32, tag="gT")
                for k in range(n_k1):
                    kk = min(128, FUSE - 128 * k)
                    ptg = hpsm.tile([128, SLOTS], f32, tag="ptx")
                    nc.tensor.transpose(
                        ptg[:kk, :], g_sb[:, 128 * k:128 * k + kk], identity[:SLOTS, :SLOTS]
                    )
                    nc.vector.tensor_copy(gT[:kk, k, :], ptg[:kk, :])
                h1ps = hpsm.tile([SLOTS, H1], f32, tag="hacc")
                for k in range(n_k1):
                    kk = min(128, FUSE - 128 * k)
                    nc.tensor.matmul(
                        h1ps, gT[:kk, k, :], rw1t[:kk, k, :],
                        start=(k == 0), stop=(k == n_k1 - 1),
                    )
                h1 = hsb.tile([SLOTS, H1], f32, tag="h1")
                nc.vector.tensor_add(h1, h1ps, rb1b)
                ln_relu(h1, H1, g1b, be1b, 1)

                # h2 = relu(LN(h1 @ rw2 + rb2))
                h1T = hsb.tile([128, 2, SLOTS], f32, tag="h1T")
                for k in range(2):
                    pth = hpsm.tile([128, SLOTS], f32, tag="ptx")
                    nc.tensor.transpose(
                        pth[:, :], h1[:, 128 * k:128 * (k + 1)], identity[:SLOTS, :SLOTS]
                    )
                    nc.vector.tensor_copy(h1T[:, k, :], pth[:, :])
                h2ps = hpsm.tile([SLOTS, H2], f32, tag="hacc")
                for k in range(2):
                    nc.tensor.matmul(
                        h2ps, h1T[:, k, :], rw2t[:, k, :],
                        start=(k == 0), stop=(k == 1),
                    )
                h2 = hsb.tile([SLOTS, H2], f32, tag="h2")
                nc.vector.tensor_add(h2, h2ps, rb2b)
                ln_relu(h2, H2, g2b, be2b, 2)

                # out = h2 @ rw3 + rb3
                h2T = hsb.tile([128, SLOTS], f32, tag="h2T")
                pto = hpsm.tile([128, SLOTS], f32, tag="ptx")
                nc.tensor.transpose(pto[:, :], h2[:, :], identity[:SLOTS, :SLOTS])
                nc.vector.tensor_copy(h2T, pto)
                ops_ = hpsm.tile([SLOTS, NOUT], f32, tag="hacc")
                nc.tensor.matmul(ops_, h2T, rw3t, start=True, stop=True)
                osb = hsb.tile([SLOTS, NOUT], f32, tag="osb")
                nc.vector.tensor_add(osb, ops_, rb3b)
                nc.sync.dma_start(out=out_d[:, :], in_=osb)

    nc.compile()
    return nc


def _get_program(S_pad, masked_key):
    key = (S_pad, masked_key)
    if key not in _PROG_CACHE:
        _PROG_CACHE[key] = _build_program(S_pad, masked_key)
    return _PROG_CACHE[key]


def _segment_bounds(offset):
    off = np.asarray(offset).astype(np.int64)
    start = np.concatenate([[0], off[:-1]])
    return start, off - start


def _core_feat(feat, start, lens, core, S_pad):
    """Rows for this core's SLOTS segments, padded to [SLOTS * S_pad, C]."""
    segs = range(core * SLOTS, (core + 1) * SLOTS)
    if all(
        lens[j] == S_pad and start[j] == j * S_pad
        for j in segs
    ):
        return feat[core * SLOTS * S_pad:(core + 1) * SLOTS * S_pad]
    buf = np.zeros((SLOTS * S_pad, C), np.float32)
    for s, j in enumerate(segs):
        L = int(lens[j])
        if L > 0:
            buf[s * S_pad:s * S_pad + L] = feat[int(start[j]):int(start[j]) + L]
    return buf


def _prepare(inputs):
    lfeat = np.ascontiguousarray(np.asarray(inputs["local_feat"]), dtype=np.float32)
    pfeat = np.ascontiguousarray(np.asarray(inputs["parent_feat"]), dtype=np.float32)
    lstart, llen = _segment_bounds(inputs["local_offset"])
    pstart, plen = _segment_bounds(inputs["parent_offset"])
    cid = np.asarray(inputs["category_id"]).astype(np.int64)
    cat_emb = np.asarray(inputs["cat_emb"], dtype=np.float32)
    assert llen.shape[0] == B and plen.shape[0] == B

    S_pad = int(max(llen.max(), plen.max(), 1))
    S_pad = ((S_pad + 127) // 128) * 128
    NCH = S_pad // 128

    masked = set()
    for t, lens in ((0, llen), (1, plen)):
        for j in range(B):
            L = int(lens[j])
            slot = j % SLOTS
            for ch in range(NCH):
                v = min(128, max(0, L - ch * 128))
                if v < 128:
                    masked.add((t, slot, ch))
    masked_key = tuple(sorted(masked))

    weights = {k: np.ascontiguousarray(np.asarray(inputs[k]), dtype=np.float32)
               for k in _WNAMES}
    in_maps = []
    for core in range(NCORES):
        m = dict(weights)
        m["lf"] = _core_feat(lfeat, lstart, llen, core, S_pad)
        m["pf"] = _core_feat(pfeat, pstart, plen, core, S_pad)
        m["emb"] = np.ascontiguousarray(
            cat_emb[cid[core * SLOTS:(core + 1) * SLOTS]], dtype=np.float32)
        if masked_key:
            rows = np.zeros((len(masked_key), 128), np.float32)
            for i, (t, slot, ch) in enumerate(masked_key):
                j = core * SLOTS + slot
                L = int((llen if t == 0 else plen)[j])
                v = min(128, max(0, L - ch * 128))
                rows[i, :v] = 1.0
            m["maskrows"] = rows
        in_maps.append(m)
    return S_pad, masked_key, in_maps


def _ensure_ntff_hook():
    """Register the axon NTFF profiling hook if the antenv shim module is
    missing (trimmed container). Profiling-only; the grading path never needs it."""
    import types
    try:
        from antenv import axon_hooks  # noqa: F401
        return
    except ImportError:
        pass
    try:
        import antenv
        mod = types.ModuleType("antenv.axon_hooks")
        _h = {"hook": None}
        mod.set_axon_ntff_profile_hook = lambda h: _h.__setitem__("hook", h)
        mod.get_axon_ntff_profile_hook = lambda: _h["hook"]
        sys.modules["antenv.axon_hooks"] = mod
        antenv.axon_hooks = mod
        from trn_agent_boot.trn_boot import _ntff_profile_via_ctypes
        hook = _ntff_profile_via_ctypes("/opt/axon/libaxon_pjrt.so")
        if hook is not None:
            mod.set_axon_ntff_profile_hook(hook)
    except Exception:
        pass


def _run_inproc(inputs, trace=False):
    from concourse import bass_utils
    if trace:
        _ensure_ntff_hook()
    S_pad, masked_key, in_maps = _prepare(inputs)
    nc = _get_program(S_pad, masked_key)
    bkr = bass_utils.run_bass_kernel_spmd(
        nc, in_maps, core_ids=list(range(NCORES)), trace=trace)
    out = np.concatenate(
        [np.asarray(r["out"], np.float32).reshape(SLOTS, NOUT) for r in bkr.results],
        axis=0,
    )
    return out, bkr


def _jax_platform_ok():
    """The device run needs a non-CPU (axon/neuron) PJRT backend in-process."""
    if "jax" not in sys.modules:
        os.environ.setdefault("JAX_PLATFORMS", "axon")
    try:
        import jax
        return any(d.platform != "cpu" for d in jax.devices())
    except Exception:
        return False


def _run_subproc(inputs):
    """Fallback: run the device part in a child process with the axon backend."""
    with tempfile.TemporaryDirectory() as td:
        inp = os.path.join(td, "in.npz")
        outp = os.path.join(td, "out.npy")
        np.savez(inp, **{k: np.asarray(v) for k, v in inputs.items()})
        driver = (
            "import os, sys, numpy as np\n"
            f"sys.path.insert(0, {os.path.dirname(os.path.abspath(__file__))!r})\n"
            "import kernel as K\n"
            f"d = dict(np.load({inp!r}))\n"
            f"out, _ = K._run_inproc(d)\n"
            f"np.save({outp!r}, out)\n"
        )
        env = dict(os.environ)
        env["JAX_PLATFORMS"] = "axon"
        subprocess.run([sys.executable, "-c", driver], env=env, check=True)
        return np.load(outp)


def kernel(**inputs):
    # The subprocess path cannot recurse: the child calls _run_inproc directly.
    # It exists only for host processes whose jax is pinned to CPU.
    if _jax_platform_ok():
        return _run_inproc(inputs)[0]
    return _run_subproc(inputs)


if __name__ == "__main__":
    # smoke test with tiny random data (ragged offsets exercise the mask path)
    rng = np.random.default_rng(0)
    Npts = 16 * 320
    off = np.sort(rng.choice(np.arange(1, Npts), size=B - 1, replace=False))
    off = np.concatenate([off, [Npts]]).astype(np.int32)
    inp = {
        "local_feat": rng.standard_normal((Npts, C), dtype=np.float32),
        "parent_feat": rng.standard_normal((Npts, C), dtype=np.float32),
        "local_offset": off,
        "parent_offset": off,
        "category_id": rng.integers(0, 3, B).astype(np.int32),
        "att_w1": 0.02 * rng.standard_normal((C, H), dtype=np.float32),
        "att_b1": 0.01 * rng.standard_normal(H).astype(np.float32),
        "att_w2": 0.02 * rng.standard_normal(H).astype(np.float32),
        "patt_w1": 0.02 * rng.standard_normal((C, H), dtype=np.float32),
        "patt_b1": 0.01 * rng.standard_normal(H).astype(np.float32),
        "patt_w2": 0.02 * rng.standard_normal(H).astype(np.float32),
        "cat_emb": rng.standard_normal((3, EMB), dtype=np.float32),
        "rw1": 0.02 * rng.standard_normal((FUSE, H1), dtype=np.float32),
        "rb1": 0.01 * rng.standard_normal(H1).astype(np.float32),
        "ln1_g": (1 + 0.1 * rng.standard_normal(H1)).astype(np.float32),
        "ln1_b": 0.1 * rng.standard_normal(H1).astype(np.float32),
        "rw2": 0.02 * rng.standard_normal((H1, H2), dtype=np.float32),
        "rb2": 0.01 * rng.standard_normal(H2).astype(np.float32),
        "ln2_g": (1 + 0.1 * rng.standard_normal(H2)).astype(np.float32),
        "ln2_b": 0.1 * rng.standard_normal(H2).astype(np.float32),
        "rw3": 0.02 * rng.standard_normal((H2, NOUT), dtype=np.float32),
        "rb3": 0.01 * rng.standard_normal(NOUT).astype(np.float32),
    }
    got = kernel(**inp)

    # numpy reference
    def np_ref(i):
        def pool(feat, off, w1, b1, w2):
            start = 0
            outs = []
            for j in range(B):
                seg = feat[start:off[j]]
                start = off[j]
                s = np.maximum(seg @ w1 + b1, 0) @ w2
                e = np.exp(s - (s.max() if s.size else 0.0))
                outs.append((seg * e[:, None]).sum(0) / max(e.sum(), 1e-37)
                            if s.size else np.zeros(C, np.float32))
            return np.stack(outs)

        lg = pool(i["local_feat"], i["local_offset"], i["att_w1"], i["att_b1"], i["att_w2"])
        pg = pool(i["parent_feat"], i["parent_offset"], i["patt_w1"], i["patt_b1"], i["patt_w2"])
        g = np.concatenate([lg, pg, i["cat_emb"][i["category_id"]]], -1)

        def lnr(x, gm, bt):
            mu = x.mean(-1, keepdims=True)
            v = x.var(-1, keepdims=True)
            return np.maximum((x - mu) / np.sqrt(v + LN_EPS) * gm + bt, 0)

        h = lnr(g @ i["rw1"] + i["rb1"], i["ln1_g"], i["ln1_b"])
        h = lnr(h @ i["rw2"] + i["rb2"], i["ln2_g"], i["ln2_b"])
        return h @ i["rw3"] + i["rb3"]

    want = np_ref(inp)
    err = np.max(np.abs(got - want)) / max(np.max(np.abs(want)), 1e-30)
    print("smoke got[0]:", got[0], " want[0]:", want[0])
    print("smoke rel err:", err)
